# revision 30
# baseline (speedup 1.0000x reference)
"""DGCNN feature extractor on 8 Trainium2 NeuronCores (Bass/Tile).

Strategy: data-parallel over batch B=8 (one sample per core).
Per layer (edge-conv):
  - scores s[n,m] = <x_n, x_m> - |x_m|^2/2 (rank-equivalent to the
    reference's -pairwise-distance) via THREE bf16 matmuls per 1024-col
    chunk using a hi/lo bf16 split (x = hi + lo):
    s = hiq.hik + hiq.lok + loq.hik accumulated in fp32 PSUM, where the
    q-side aug row is ones (lo = 0, term dropped) and the k-side aug row
    is the hi/lo split of -|x_m|^2/2. Empirically (numpy study) this
    keeps final rel err ~1.4e-3 (vs 2e-2 budget); single-bf16 scores or
    reduced-precision selection fail. bf16 matmul streams 1 col/cycle
    vs fp32's 4, cutting PE time ~2.3x.
  - selection stays fp32: PSUM copied to a full-width fp32 SBUF row;
    chunked max8 (8x512) -> top-16 of 64 candidates -> max_index over
    the fp32 row for global indices.
  - conv decomposes as y[o,n,k] = u[o,n] + v[o, idx[n,k]] with
    u = (A-B)x, v = Bx (W = [A|B] split, host-prepped); 16 per-k
    vector-indirect DMAs gather v rows (SWDGE supports one offset per
    partition per instruction - batched offsets verified broken on HW).
    GpSimd does ONLY gathers + collectives: all other work was moved to
    PE/DVE/ACT so the gather descriptor generation (~1.32us x 16/tile),
    which is the hard architectural floor here, is the sole GpSimd load.
  - k-folds are contiguous in-place log-trees (max on g, sum on a small
    scratch, sum-sq on g2), replacing 4x-slower strided reduces.
  - BN batch stats fused per tile into two SBUF accumulators:
    s1 += 16u + sum_k v ; s2 += 16u^2 + 2u*sum_k v + sum_k v^2, then one
    PE ones-matmul pair -> [O,2] -> AllReduce across the 8 cores while
    the (u+D) transposes run; x_next = relu(a*(u + max_k v) + b) since
    ReLU(LeakyReLU(z)) = ReLU(z) and the BN scale is positive.
Final: per-channel max over points, concat 32+32+64, FC on device, host
stacks the 8 per-core [64] outputs.
"""
import numpy as np

B, C0, N, KNB = 8, 3, 4096, 16
O1, O2, O3 = 32, 32, 64
NCORES = 8
EPS = 1e-5
NTOT = float(B * N * KNB)
NT = N // 128          # 32 point-tiles per layer
NCH = 8                # score chunks per row (4096/512)
CHK = N // NCH         # 512
NEG = -3.0e38

_cache: dict = {}


def _build(sim_single=False, use_collective=True, ssb_bufs=2, nquart=4):
    import concourse.bacc as bacc
    import concourse.bass as bass
    import concourse.mybir as mybir
    import concourse.tile as tile
    from concourse.masks import make_identity

    f32 = mybir.dt.float32
    bf16 = mybir.dt.bfloat16
    u32 = mybir.dt.uint32
    AO = mybir.AluOpType
    AF = mybir.ActivationFunctionType

    nc = bacc.Bacc("TRN2", target_bir_lowering=False, debug=False,
                   num_devices=1 if sim_single else NCORES)

    # ---- I/O ----
    x_in = nc.dram_tensor("x", [C0, N], f32, kind="ExternalInput")
    wuv_in = [None,
              nc.dram_tensor("wuv1", [C0, 2 * O1], f32, kind="ExternalInput"),
              nc.dram_tensor("wuv2", [O1, 2 * O2], f32, kind="ExternalInput"),
              nc.dram_tensor("wuv3", [O2, 2 * O3], f32, kind="ExternalInput")]
    gb_in = [None,
             nc.dram_tensor("gb1", [O1, 2], f32, kind="ExternalInput"),
             nc.dram_tensor("gb2", [O2, 2], f32, kind="ExternalInput"),
             nc.dram_tensor("gb3", [O3, 2], f32, kind="ExternalInput")]
    wfct_in = nc.dram_tensor("wfct", [128, 64], f32, kind="ExternalInput")
    bfc_in = nc.dram_tensor("bfc", [1, 64], f32, kind="ExternalInput")
    onesbf_in = nc.dram_tensor("onesbf", [1, N], mybir.dt.bfloat16,
                               kind="ExternalInput")
    out_d = nc.dram_tensor("out", [1, 64], f32, kind="ExternalOutput")

    # ---- internal DRAM ----
    vtab = [None,
            nc.dram_tensor("vtab1", [N, O1], f32),
            nc.dram_tensor("vtab2", [N, O2], f32),
            nc.dram_tensor("vtab3", [N, O3], f32)]
    cc_in = [None] + [nc.dram_tensor(f"ccin{l}", [o, 2], f32)
                      for l, o in ((1, O1), (2, O2), (3, O3))]
    cc_out = [None] + [nc.dram_tensor(f"ccout{l}", [o, 2], f32,
                                      addr_space="Shared")
                       for l, o in ((1, O1), (2, O2), (3, O3))]

    QW = N // nquart       # 1024 columns per score chunk

    with tile.TileContext(nc) as tc:
        with (
            tc.tile_pool(name="big", bufs=2) as bigp,        # xq generations
            tc.tile_pool(name="lay", bufs=1) as layp,        # per-layer buffers
            tc.tile_pool(name="work", bufs=3) as workp,      # small per-tile tiles
            tc.tile_pool(name="ssb", bufs=ssb_bufs) as ssbp, # SBUF score rows
            tc.tile_pool(name="gpool", bufs=3) as gp,
            tc.tile_pool(name="ugp", bufs=2) as ugp,        # gather tiles
            tc.tile_pool(name="const", bufs=1) as constp,
        ):
            ident = constp.tile([128, 128], f32)
            make_identity(nc, ident[:])
            ones128 = constp.tile([128, 1], f32)
            nc.vector.memset(ones128[:], 1.0)
            zero128 = constp.tile([128, 1], f32)
            nc.vector.memset(zero128[:], 0.0)
            eps128 = constp.tile([128, 1], f32)
            nc.vector.memset(eps128[:], EPS)
            xg = constp.tile([128, 1], f32)   # pooled channel maxes (x1|x2|x3)

            def layer(l, C, O, xq, is_last):
                """xq: [C, N] fp32 SBUF tile (features only).
                Returns next layer's xq ([O, N] fp32) or None if is_last."""
                # --- phase A: sq row, bf16 hi/lo gens, u/v matmuls ---
                wuv = constp.tile([C, 2 * O], f32, tag=f"wuv{l}")
                nc.sync.dma_start(out=wuv[:], in_=wuv_in[l].ap())
                gb = constp.tile([O, 2], f32, tag=f"gb{l}")
                nc.sync.dma_start(out=gb[:], in_=gb_in[l].ap())
                onesC = constp.tile([C, 1], f32, tag=f"onesC{l}")
                nc.vector.memset(onesC[:], 1.0)

                ubuf = layp.tile([128, NT, O], f32, tag="ubuf")
                # bf16 hi/lo. q: rows 0..C-1 features, row C ones (lo term
                # dropped since lo(ones)=0). k: row C = hi/lo of -|x_m|^2/2.
                hiq = layp.tile([C + 1, N], bf16, tag="hiq")
                loq = layp.tile([C, N], bf16, tag="loq")
                hik = layp.tile([C + 1, N], bf16, tag="hik")
                lok = layp.tile([C + 1, N], bf16, tag="lok")

                with tc.tile_pool(name=f"psA{l}", bufs=2, space="PSUM") as psA:
                    for ch in range(8):
                        sl = slice(512 * ch, 512 * (ch + 1))
                        xsq = workp.tile([C, 512], f32, tag="xsq")
                        nc.scalar.activation(out=xsq[:], in_=xq[0:C, sl],
                                             func=AF.Square, bias=zero128[0:C, :])
                        sq_ps = psA.tile([1, 512], f32, tag="sqps")
                        nc.tensor.matmul(out=sq_ps[:], lhsT=onesC[:],
                                         rhs=xsq[:], start=True, stop=True)
                        sqf = workp.tile([1, 512], f32, tag="sqf")
                        nc.scalar.activation(out=sqf[:], in_=sq_ps[:],
                                             func=AF.Copy, scale=-0.5)
                        sqh = workp.tile([1, 512], bf16, tag="sqh")
                        nc.scalar.activation(out=sqh[:], in_=sqf[:],
                                             func=AF.Copy)
                        sql = workp.tile([1, 512], bf16, tag="sql")
                        nc.vector.tensor_sub(sql[:], sqf[:], sqh[:])
                        nc.sync.dma_start(out=hik[C:C + 1, sl], in_=sqh[:])
                        nc.sync.dma_start(out=lok[C:C + 1, sl], in_=sql[:])
                    # hi/lo feature rows (ACT cast + DVE subtract), in
                    # column chunks so early score quarters start sooner
                    nc.sync.dma_start(out=hiq[C:C + 1, :], in_=onesbf_in.ap())
                    for qh in range(4):
                        qs = slice(1024 * qh, 1024 * (qh + 1))
                        nc.scalar.activation(out=hiq[0:C, qs], in_=xq[0:C, qs],
                                             func=AF.Copy)
                        nc.vector.tensor_sub(loq[0:C, qs], xq[0:C, qs],
                                             hiq[0:C, qs])
                        nc.sync.dma_start(out=hik[0:C, qs], in_=hiq[0:C, qs])
                        nc.sync.dma_start(out=lok[0:C, qs], in_=loq[0:C, qs])

                    for tu in range(NT):
                        tlu = slice(128 * tu, 128 * (tu + 1))
                        uv_ps = psA.tile([128, 2 * O], f32, tag="uvps")
                        nc.tensor.matmul(out=uv_ps[:], lhsT=xq[0:C, tlu],
                                         rhs=wuv[:], start=True, stop=True)
                        nc.scalar.activation(out=ubuf[:, tu, :],
                                             in_=uv_ps[:, 0:O], func=AF.Copy)
                        vstage = workp.tile([128, O], f32, tag="vstage")
                        nc.scalar.activation(out=vstage[:],
                                             in_=uv_ps[:, O:2 * O],
                                             func=AF.Copy)
                        nc.sync.dma_start(out=vtab[l].ap()[tlu, :],
                                          in_=vstage[:])


                # u and u^2 sums over tiles (needs only ubuf; overlaps phase B)
                usum = layp.tile([128, NT // 2, O], f32, tag="usum")
                nc.vector.tensor_tensor(out=usum[:], in0=ubuf[:, 0:NT // 2, :],
                                        in1=ubuf[:, NT // 2:NT, :], op=AO.add)
                usq = layp.tile([128, NT // 2, O], f32, tag="usq")
                nc.scalar.activation(out=usq[:], in_=ubuf[:, 0:NT // 2, :],
                                     func=AF.Square)
                usq2 = workp.tile([128, NT // 2, O], f32, tag="usq2")
                nc.scalar.activation(out=usq2[:], in_=ubuf[:, NT // 2:NT, :],
                                     func=AF.Square)
                nc.vector.tensor_tensor(out=usq[:], in0=usq[:], in1=usq2[:],
                                        op=AO.add)
                for hh in (8, 4, 2, 1):
                    nc.vector.tensor_tensor(out=usum[:, 0:hh, :],
                                            in0=usum[:, 0:hh, :],
                                            in1=usum[:, hh:2 * hh, :], op=AO.add)
                    nc.vector.tensor_tensor(out=usq[:, 0:hh, :],
                                            in0=usq[:, 0:hh, :],
                                            in1=usq[:, hh:2 * hh, :], op=AO.add)

                # BN stat accumulators (summed over tiles, fp32, k-resolved)
                s1a = layp.tile([128, KNB, O], f32, tag="s1a")
                qa = layp.tile([128, KNB, O], f32, tag="qa")
                uga = layp.tile([128, KNB, O], f32, tag="uga")
                nc.vector.memset(s1a[:], 0.0)
                nc.vector.memset(qa[:], 0.0)
                nc.vector.memset(uga[:], 0.0)

                # --- phase B: scores + topk + gather + folds ---
                # (tile 0's score matmuls emitted before the u/v matmuls so
                # the PE pipeline restarts immediately at the layer boundary;
                # all vtab rows still land before the first gather)
                Dbuf = layp.tile([128, NT, O], f32, tag="Dbuf")
                with tc.tile_pool(name=f"psB{l}", bufs=2, space="PSUM") as psB:
                  def emit_scores(t, ssb):
                    tl = slice(128 * t, 128 * (t + 1))
                    for h in range(nquart):
                        sps = psB.tile([128, QW], f32, tag="sps")
                        for sub in range(QW // 512):
                            oo = slice(512 * sub, 512 * (sub + 1))
                            so = slice(QW * h + 512 * sub,
                                       QW * h + 512 * (sub + 1))
                            nc.tensor.matmul(out=sps[:, oo], lhsT=hiq[:, tl],
                                             rhs=hik[:, so],
                                             start=True, stop=False)
                            nc.tensor.matmul(out=sps[:, oo], lhsT=hiq[:, tl],
                                             rhs=lok[:, so],
                                             start=False, stop=False)
                            nc.tensor.matmul(out=sps[:, oo], lhsT=loq[:, tl],
                                             rhs=hik[0:C, so],
                                             start=False, stop=True)
                        nc.scalar.activation(out=ssb[:, QW * h:QW * (h + 1)],
                                             in_=sps[:], func=AF.Copy)

                  for t in range(NT):
                    tl = slice(128 * t, 128 * (t + 1))
                    ssb = ssbp.tile([128, N], f32, tag="ssb")
                    emit_scores(t, ssb)
                    cand = workp.tile([128, 8 * NCH], f32, tag="cand")
                    for ch in range(NCH):
                        nc.vector.max(out=cand[:, 8 * ch:8 * ch + 8],
                                      in_=ssb[:, CHK * ch:CHK * (ch + 1)])
                    t16 = workp.tile([128, 16], f32, tag="t16")
                    cand2 = workp.tile([128, 8 * NCH], f32, tag="cand2")
                    nc.vector.max(out=t16[:, 0:8], in_=cand[:])
                    nc.vector.match_replace(out=cand2[:], in_to_replace=t16[:, 0:8],
                                            in_values=cand[:], imm_value=NEG)
                    nc.vector.max(out=t16[:, 8:16], in_=cand2[:])
                    idxs = workp.tile([128, 16], u32, tag="idxs")
                    nc.vector.max_index(out=idxs[:, 0:8], in_max=t16[:, 0:8],
                                        in_values=ssb[:])
                    nc.vector.max_index(out=idxs[:, 8:16], in_max=t16[:, 8:16],
                                        in_values=ssb[:])
                    # 16 indirect gathers (vector-indirect: 1 offset/partition)
                    g = gp.tile([128, KNB, O], f32, tag="g")
                    for k in range(KNB):
                        nc.gpsimd.indirect_dma_start(
                            out=g[:, k, :], out_offset=None, in_=vtab[l].ap(),
                            in_offset=bass.IndirectOffsetOnAxis(
                                ap=idxs[:, k:k + 1], axis=0))
                    g2 = gp.tile([128, KNB, O], f32, tag="g2")
                    nc.scalar.activation(out=g2[:], in_=g[:], func=AF.Square)
                    nc.vector.tensor_tensor(out=qa[:], in0=qa[:], in1=g2[:],
                                            op=AO.add)
                    nc.vector.tensor_tensor(out=s1a[:], in0=s1a[:], in1=g[:],
                                            op=AO.add)
                    ubc = ubuf[:, t, :]
                    ubc = bass.AP(ubc.tensor, ubc.offset,
                                  [ubc.ap[0], [0, KNB], ubc.ap[-1]])
                    ug16 = ugp.tile([128, KNB, O], f32, tag="ug16")
                    nc.vector.tensor_tensor(out=ug16[:], in0=g[:, :, :],
                                            in1=ubc, op=AO.mult)
                    nc.vector.tensor_tensor(out=uga[:], in0=uga[:], in1=ug16[:],
                                            op=AO.add)
                    # max tree in place on g; z = u + max_k v into Dbuf
                    nc.vector.tensor_tensor(out=g[:, 0:8, :], in0=g[:, 0:8, :],
                                            in1=g[:, 8:16, :], op=AO.max)
                    for hh in (4, 2):
                        nc.vector.tensor_tensor(
                            out=g[:, 0:hh, :], in0=g[:, 0:hh, :],
                            in1=g[:, hh:2 * hh, :], op=AO.max)
                    nc.vector.tensor_tensor(out=Dbuf[:, t, :], in0=g[:, 0, :],
                                            in1=g[:, 1, :], op=AO.max)
                    nc.vector.tensor_tensor(out=Dbuf[:, t, :], in0=Dbuf[:, t, :],
                                            in1=ubuf[:, t, :], op=AO.add)
                # --- phase C: stats, allreduce (overlapped with transposes) ---
                # fold accumulators over k; build S1/S2; reduce over p via PE
                for hh in (8, 4, 2):
                    for acc in (s1a, qa, uga):
                        nc.vector.tensor_tensor(
                            out=acc[:, 0:hh, :], in0=acc[:, 0:hh, :],
                            in1=acc[:, hh:2 * hh, :], op=AO.add)
                # S1 = Gv + 16u (into s1a[:,0,:] + s1a[:,1,:] pre-fold)
                s1f = workp.tile([128, O], f32, tag="s1f")
                nc.vector.tensor_tensor(out=s1f[:], in0=s1a[:, 0, :],
                                        in1=s1a[:, 1, :], op=AO.add)
                nc.vector.scalar_tensor_tensor(
                    out=s1f[:], in0=usum[:, 0, :], scalar=16.0,
                    in1=s1f[:], op0=AO.mult, op1=AO.add)
                # S2 = Gss + 2*uGv + 16*usq
                s2f = workp.tile([128, O], f32, tag="s2f")
                nc.vector.tensor_tensor(out=s2f[:], in0=qa[:, 0, :],
                                        in1=qa[:, 1, :], op=AO.add)
                ugf = workp.tile([128, O], f32, tag="ugf")
                nc.vector.tensor_tensor(out=ugf[:], in0=uga[:, 0, :],
                                        in1=uga[:, 1, :], op=AO.add)
                nc.vector.scalar_tensor_tensor(
                    out=s2f[:], in0=ugf[:], scalar=2.0,
                    in1=s2f[:], op0=AO.mult, op1=AO.add)
                nc.vector.scalar_tensor_tensor(
                    out=s2f[:], in0=usq[:, 0, :], scalar=16.0,
                    in1=s2f[:], op0=AO.mult, op1=AO.add)
                with tc.tile_pool(name=f"psR{l}", bufs=1, space="PSUM") as psR:
                    s_ps = psR.tile([O, 2], f32, tag="sps2")
                    nc.tensor.matmul(out=s_ps[:, 0:1], lhsT=s1f[:],
                                     rhs=ones128[:], start=True, stop=True)
                    nc.tensor.matmul(out=s_ps[:, 1:2], lhsT=s2f[:],
                                     rhs=ones128[:], start=True, stop=True)
                    stg = workp.tile([O, 2], f32, tag="stg")
                    nc.vector.tensor_copy(out=stg[:], in_=s_ps[:])
                    nc.sync.dma_start(out=cc_in[l].ap(), in_=stg[:])
                if sim_single or not use_collective:
                    nc.sync.dma_start(out=cc_out[l].ap(), in_=cc_in[l].ap())
                else:
                    nc.gpsimd.collective_compute(
                        "AllReduce", AO.add, replica_groups=[list(range(NCORES))],
                        ins=[cc_in[l].ap()], outs=[cc_out[l].ap()])

                # transposes into xnq overlap the collective (plain copies)
                xnq = bigp.tile([O3 if is_last else O, N], f32, tag="xq")
                with tc.tile_pool(name=f"psT{l}", bufs=4, space="PSUM") as psT:
                    for t in range(NT):
                        tl = slice(128 * t, 128 * (t + 1))
                        tp = psT.tile([O, 128], f32, tag="tpps")
                        nc.tensor.transpose(out=tp[:], in_=Dbuf[:, t, :],
                                            identity=ident[:])
                        nc.scalar.activation(out=xnq[0:O, tl], in_=tp[:],
                                             func=AF.Copy)

                # stats arrive: finish BN affine coefficients
                stats = workp.tile([O, 2], f32, tag="stats")
                nc.sync.dma_start(out=stats[:], in_=cc_out[l].ap())
                mean = workp.tile([O, 4], f32, tag="mean")
                nc.vector.tensor_scalar(mean[:, 0:1], stats[:, 0:1], 1.0 / NTOT,
                                        None, op0=AO.mult)
                nc.vector.tensor_scalar(mean[:, 1:2], stats[:, 1:2], 1.0 / NTOT,
                                        None, op0=AO.mult)
                # var = E2 - mean^2 ; sd = sqrt(var+eps); a = gamma/sd; b = beta - mean*a
                nc.vector.tensor_tensor(out=mean[:, 2:3], in0=mean[:, 0:1],
                                        in1=mean[:, 0:1], op=AO.mult)
                nc.vector.tensor_sub(mean[:, 1:2], mean[:, 1:2], mean[:, 2:3])
                nc.scalar.activation(out=mean[:, 1:2], in_=mean[:, 1:2],
                                     func=AF.Sqrt, bias=eps128[0:O, :])
                ab = workp.tile([O, 2], f32, tag="ab")
                nc.vector.reciprocal(out=ab[:, 0:1], in_=mean[:, 1:2])
                nc.vector.tensor_tensor(out=ab[:, 0:1], in0=ab[:, 0:1],
                                        in1=gb[:, 0:1], op=AO.mult)
                nc.vector.tensor_tensor(out=mean[:, 3:4], in0=mean[:, 0:1],
                                        in1=ab[:, 0:1], op=AO.mult)
                nc.vector.tensor_sub(ab[:, 1:2], gb[:, 1:2], mean[:, 3:4])

                # x_next = relu(a*(u+D) + b), column-chunked so next-layer
                # phase A pipelines behind it
                for qh in range(4):
                    qs = slice(1024 * qh, 1024 * (qh + 1))
                    nc.scalar.activation(out=xnq[0:O, qs], in_=xnq[0:O, qs],
                                         func=AF.Relu,
                                         bias=ab[:, 1:2], scale=ab[:, 0:1])
                # channel max pool -> xg slice
                cm = workp.tile([O, 1], f32, tag="cm")
                nc.vector.tensor_reduce(out=cm[:], in_=xnq[0:O, :],
                                        axis=mybir.AxisListType.X, op=AO.max)
                off = {1: 0, 2: O1, 3: O1 + O2}[l]
                nc.sync.dma_start(out=xg[off:off + O, :], in_=cm[:])
                return xnq

            # layer 1 input
            xq1 = bigp.tile([C0, N], f32, tag="xq")
            nc.sync.dma_start(out=xq1[0:C0, :], in_=x_in.ap())

            xq2 = layer(1, C0, O1, xq1, False)
            xq3 = layer(2, O1, O2, xq2, False)
            layer(3, O2, O3, xq3, True)

            # FC: out = xg^T @ WfcT + bfc
            wfct = constp.tile([128, 64], f32)
            nc.sync.dma_start(out=wfct[:], in_=wfct_in.ap())
            bfc = constp.tile([1, 64], f32)
            nc.sync.dma_start(out=bfc[:], in_=bfc_in.ap())
            with tc.tile_pool(name="psF", bufs=1, space="PSUM") as psF:
                fc_ps = psF.tile([1, 64], f32, tag="fcps")
                nc.tensor.matmul(out=fc_ps[:], lhsT=xg[:], rhs=wfct[:],
                                 start=True, stop=True)
                ores = constp.tile([1, 64], f32)
                nc.vector.tensor_add(ores[:], fc_ps[:], bfc[:])
                nc.sync.dma_start(out=out_d.ap(), in_=ores[:])

    nc.compile()
    return nc


def _get_nc():
    if "nc" not in _cache:
        _cache["nc"] = _build()
    return _cache["nc"]


def _bf16_dtype():
    from ml_dtypes import bfloat16  # ships with jax
    return bfloat16


def _prep_inputs(x, W1, g1, b1, W2, g2, b2, W3, g3, b3, Wfc, bfc):
    """Host-side weight prep -> per-core input maps."""
    def wuv(W, C):
        A, Bm = W[:, :C], W[:, C:]
        return np.concatenate([(A - Bm).T, Bm.T], axis=1).astype(np.float32)

    common = {
        "wuv1": wuv(np.asarray(W1), C0),
        "wuv2": wuv(np.asarray(W2), O1),
        "wuv3": wuv(np.asarray(W3), O2),
        "gb1": np.stack([g1, b1], 1).astype(np.float32),
        "gb2": np.stack([g2, b2], 1).astype(np.float32),
        "gb3": np.stack([g3, b3], 1).astype(np.float32),
        "wfct": np.asarray(Wfc).T.copy().astype(np.float32),
        "onesbf": np.ones((1, N), np.float32).astype(_bf16_dtype()),
        "bfc": np.asarray(bfc)[None, :].astype(np.float32),
    }
    x = np.asarray(x, dtype=np.float32)
    return [{**common, "x": np.ascontiguousarray(x[c])} for c in range(NCORES)]


def _enable_jax_cache():
    try:
        import jax
        jax.config.update("jax_compilation_cache_dir", "/tmp/jaxcache")
        jax.config.update("jax_persistent_cache_min_entry_size_bytes", -1)
        jax.config.update("jax_persistent_cache_min_compile_time_secs", 0.5)
    except Exception:
        pass


def kernel(x, W1, g1, b1, W2, g2, b2, W3, g3, b3, Wfc, bfc):
    from concourse.bass_utils import run_bass_kernel_spmd
    _enable_jax_cache()
    nc = _get_nc()
    in_maps = _prep_inputs(x, W1, g1, b1, W2, g2, b2, W3, g3, b3, Wfc, bfc)
    res = run_bass_kernel_spmd(nc, in_maps, list(range(NCORES)))
    return np.stack([res.results[c]["out"][0] for c in range(NCORES)]).astype(np.float32)


# revision 32
# speedup vs baseline: 1.0466x; 1.0466x over previous
"""DGCNN feature extractor on 8 Trainium2 NeuronCores (Bass/Tile).

Strategy: data-parallel over batch B=8 (one sample per core).
Per layer (edge-conv):
  - scores s[n,m] = <x_n, x_m> - |x_m|^2/2 (rank-equivalent to the
    reference's -pairwise-distance) via THREE bf16 matmuls per 1024-col
    chunk using a hi/lo bf16 split (x = hi + lo):
    s = hiq.hik + hiq.lok + loq.hik accumulated in fp32 PSUM, where the
    q-side aug row is ones (lo = 0, term dropped) and the k-side aug row
    is the hi/lo split of -|x_m|^2/2. Empirically (numpy study) this
    keeps final rel err ~1.4e-3 (vs 2e-2 budget); single-bf16 scores or
    reduced-precision selection fail. bf16 matmul streams 1 col/cycle
    vs fp32's 4, cutting PE time ~2.3x.
  - selection stays fp32: PSUM copied to a full-width fp32 SBUF row;
    chunked max8 (8x512) -> top-16 of 64 candidates -> max_index over
    the fp32 row for global indices.
  - conv decomposes as y[o,n,k] = u[o,n] + v[o, idx[n,k]] with
    u = (A-B)x, v = Bx (W = [A|B] split, host-prepped); 16 per-k
    vector-indirect DMAs gather v rows (SWDGE supports one offset per
    partition per instruction - batched offsets verified broken on HW).
    GpSimd does ONLY gathers + collectives: all other work was moved to
    PE/DVE/ACT so the gather descriptor generation (~1.32us x 16/tile),
    which is the hard architectural floor here, is the sole GpSimd load.
  - k-folds are contiguous in-place log-trees (max on g, sum on a small
    scratch, sum-sq on g2), replacing 4x-slower strided reduces.
  - BN batch stats fused per tile into two SBUF accumulators:
    s1 += 16u + sum_k v ; s2 += 16u^2 + 2u*sum_k v + sum_k v^2, then one
    PE ones-matmul pair -> [O,2] -> AllReduce across the 8 cores while
    the (u+D) transposes run; x_next = relu(a*(u + max_k v) + b) since
    ReLU(LeakyReLU(z)) = ReLU(z) and the BN scale is positive.
Final: per-channel max over points, concat 32+32+64, FC on device, host
stacks the 8 per-core [64] outputs.
"""
import numpy as np

B, C0, N, KNB = 8, 3, 4096, 16
O1, O2, O3 = 32, 32, 64
NCORES = 8
EPS = 1e-5
NTOT = float(B * N * KNB)
NT = N // 128          # 32 point-tiles per layer
NCH = 8                # score chunks per row (4096/512)
CHK = N // NCH         # 512
NEG = -3.0e38

_cache: dict = {}


def _build(sim_single=False, use_collective=True, ssb_bufs=3, nquart=4):
    import concourse.bacc as bacc
    import concourse.bass as bass
    import concourse.mybir as mybir
    import concourse.tile as tile
    from concourse.masks import make_identity

    f32 = mybir.dt.float32
    bf16 = mybir.dt.bfloat16
    u32 = mybir.dt.uint32
    AO = mybir.AluOpType
    AF = mybir.ActivationFunctionType

    nc = bacc.Bacc("TRN2", target_bir_lowering=False, debug=False,
                   num_devices=1 if sim_single else NCORES)

    # ---- I/O ----
    x_in = nc.dram_tensor("x", [C0, N], f32, kind="ExternalInput")
    wuv_in = [None,
              nc.dram_tensor("wuv1", [C0, 2 * O1], f32, kind="ExternalInput"),
              nc.dram_tensor("wuv2", [O1, 2 * O2], f32, kind="ExternalInput"),
              nc.dram_tensor("wuv3", [O2, 2 * O3], f32, kind="ExternalInput")]
    gb_in = [None,
             nc.dram_tensor("gb1", [O1, 2], f32, kind="ExternalInput"),
             nc.dram_tensor("gb2", [O2, 2], f32, kind="ExternalInput"),
             nc.dram_tensor("gb3", [O3, 2], f32, kind="ExternalInput")]
    wfct_in = nc.dram_tensor("wfct", [128, 64], f32, kind="ExternalInput")
    bfc_in = nc.dram_tensor("bfc", [1, 64], f32, kind="ExternalInput")
    onesbf_in = nc.dram_tensor("onesbf", [1, N], mybir.dt.bfloat16,
                               kind="ExternalInput")
    out_d = nc.dram_tensor("out", [1, 64], f32, kind="ExternalOutput")

    # ---- internal DRAM ----
    vtab = [None,
            nc.dram_tensor("vtab1", [N, O1], f32),
            nc.dram_tensor("vtab2", [N, O2], f32),
            nc.dram_tensor("vtab3", [N, O3], f32)]
    cc_in = [None] + [nc.dram_tensor(f"ccin{l}", [o, 2], f32)
                      for l, o in ((1, O1), (2, O2), (3, O3))]
    cc_out = [None] + [nc.dram_tensor(f"ccout{l}", [o, 2], f32,
                                      addr_space="Shared")
                       for l, o in ((1, O1), (2, O2), (3, O3))]

    QW = N // nquart       # 1024 columns per score chunk

    with tile.TileContext(nc) as tc:
        with (
            tc.tile_pool(name="big", bufs=2) as bigp,        # xq generations
            tc.tile_pool(name="lay", bufs=1) as layp,        # per-layer buffers
            tc.tile_pool(name="work", bufs=3) as workp,      # small per-tile tiles
            tc.tile_pool(name="ssb", bufs=ssb_bufs) as ssbp, # SBUF score rows
            tc.tile_pool(name="gpool", bufs=3) as gp,
            tc.tile_pool(name="ugp", bufs=2) as ugp,        # gather tiles
            tc.tile_pool(name="const", bufs=1) as constp,
        ):
            ident = constp.tile([128, 128], f32)
            make_identity(nc, ident[:])
            ones128 = constp.tile([128, 1], f32)
            nc.vector.memset(ones128[:], 1.0)
            zero128 = constp.tile([128, 1], f32)
            nc.vector.memset(zero128[:], 0.0)
            eps128 = constp.tile([128, 1], f32)
            nc.vector.memset(eps128[:], EPS)
            xg = constp.tile([128, 1], f32)   # pooled channel maxes (x1|x2|x3)

            def layer(l, C, O, xq, is_last):
                """xq: [C, N] fp32 SBUF tile (features only).
                Returns next layer's xq ([O, N] fp32) or None if is_last."""
                # --- phase A: sq row, bf16 hi/lo gens, u/v matmuls ---
                wuv = constp.tile([C, 2 * O], f32, tag=f"wuv{l}")
                nc.sync.dma_start(out=wuv[:], in_=wuv_in[l].ap())
                gb = constp.tile([O, 2], f32, tag=f"gb{l}")
                nc.sync.dma_start(out=gb[:], in_=gb_in[l].ap())
                onesC = constp.tile([C, 1], f32, tag=f"onesC{l}")
                nc.vector.memset(onesC[:], 1.0)

                ubuf = layp.tile([128, NT, O], f32, tag="ubuf")
                # bf16 hi/lo. q: rows 0..C-1 features, row C ones (lo term
                # dropped since lo(ones)=0). k: row C = hi/lo of -|x_m|^2/2.
                hiq = layp.tile([C + 1, N], bf16, tag="hiq")
                loq = layp.tile([C, N], bf16, tag="loq")
                hik = layp.tile([C + 1, N], bf16, tag="hik")
                lok = layp.tile([C + 1, N], bf16, tag="lok")

                with tc.tile_pool(name=f"psA{l}", bufs=2, space="PSUM") as psA:
                    for ch in range(8):
                        sl = slice(512 * ch, 512 * (ch + 1))
                        xsq = ugp.tile([C, 512], f32, tag="xsq")
                        nc.scalar.activation(out=xsq[:], in_=xq[0:C, sl],
                                             func=AF.Square, bias=zero128[0:C, :])
                        sq_ps = psA.tile([1, 512], f32, tag="sqps")
                        nc.tensor.matmul(out=sq_ps[:], lhsT=onesC[:],
                                         rhs=xsq[:], start=True, stop=True)
                        sqf = ugp.tile([1, 512], f32, tag="sqf")
                        nc.scalar.activation(out=sqf[:], in_=sq_ps[:],
                                             func=AF.Copy, scale=-0.5)
                        sqh = workp.tile([1, 512], bf16, tag="sqh")
                        nc.scalar.activation(out=sqh[:], in_=sqf[:],
                                             func=AF.Copy)
                        sql = workp.tile([1, 512], bf16, tag="sql")
                        nc.vector.tensor_sub(sql[:], sqf[:], sqh[:])
                        nc.sync.dma_start(out=hik[C:C + 1, sl], in_=sqh[:])
                        nc.sync.dma_start(out=lok[C:C + 1, sl], in_=sql[:])
                    # hi/lo feature rows (ACT cast + DVE subtract), in
                    # column chunks so early score quarters start sooner
                    nc.sync.dma_start(out=hiq[C:C + 1, :], in_=onesbf_in.ap())
                    for qh in range(4):
                        qs = slice(1024 * qh, 1024 * (qh + 1))
                        nc.scalar.activation(out=hiq[0:C, qs], in_=xq[0:C, qs],
                                             func=AF.Copy)
                        nc.vector.tensor_sub(loq[0:C, qs], xq[0:C, qs],
                                             hiq[0:C, qs])
                        nc.sync.dma_start(out=hik[0:C, qs], in_=hiq[0:C, qs])
                        nc.sync.dma_start(out=lok[0:C, qs], in_=loq[0:C, qs])

                    for tu in range(NT):
                        tlu = slice(128 * tu, 128 * (tu + 1))
                        uv_ps = psA.tile([128, 2 * O], f32, tag="uvps")
                        nc.tensor.matmul(out=uv_ps[:], lhsT=xq[0:C, tlu],
                                         rhs=wuv[:], start=True, stop=True)
                        nc.scalar.activation(out=ubuf[:, tu, :],
                                             in_=uv_ps[:, 0:O], func=AF.Copy)
                        vstage = workp.tile([128, O], f32, tag="vstage")
                        nc.scalar.activation(out=vstage[:],
                                             in_=uv_ps[:, O:2 * O],
                                             func=AF.Copy)
                        nc.sync.dma_start(out=vtab[l].ap()[tlu, :],
                                          in_=vstage[:])


                # u and u^2 sums over tiles (needs only ubuf; overlaps phase B)
                usum = layp.tile([128, NT // 2, O], f32, tag="usum")
                nc.vector.tensor_tensor(out=usum[:], in0=ubuf[:, 0:NT // 2, :],
                                        in1=ubuf[:, NT // 2:NT, :], op=AO.add)
                usq = layp.tile([128, NT // 2, O], f32, tag="usq")
                nc.scalar.activation(out=usq[:], in_=ubuf[:, 0:NT // 2, :],
                                     func=AF.Square)
                usq2 = layp.tile([128, NT // 2, O], f32, tag="usq2")
                nc.scalar.activation(out=usq2[:], in_=ubuf[:, NT // 2:NT, :],
                                     func=AF.Square)
                nc.vector.tensor_tensor(out=usq[:], in0=usq[:], in1=usq2[:],
                                        op=AO.add)
                for hh in (8, 4, 2, 1):
                    nc.vector.tensor_tensor(out=usum[:, 0:hh, :],
                                            in0=usum[:, 0:hh, :],
                                            in1=usum[:, hh:2 * hh, :], op=AO.add)
                    nc.vector.tensor_tensor(out=usq[:, 0:hh, :],
                                            in0=usq[:, 0:hh, :],
                                            in1=usq[:, hh:2 * hh, :], op=AO.add)

                # BN stat accumulators (summed over tiles, fp32, k-resolved)
                s1a = layp.tile([128, KNB, O], f32, tag="s1a")
                qa = layp.tile([128, KNB, O], f32, tag="qa")
                uga = layp.tile([128, KNB, O], f32, tag="uga")
                nc.vector.memset(s1a[:], 0.0)
                nc.vector.memset(qa[:], 0.0)
                nc.vector.memset(uga[:], 0.0)

                # --- phase B: scores + topk + gather + folds ---
                # (tile 0's score matmuls emitted before the u/v matmuls so
                # the PE pipeline restarts immediately at the layer boundary;
                # all vtab rows still land before the first gather)
                Dbuf = layp.tile([128, NT, O], f32, tag="Dbuf")
                with tc.tile_pool(name=f"psB{l}", bufs=2, space="PSUM") as psB:
                  def emit_scores(t, ssb):
                    tl = slice(128 * t, 128 * (t + 1))
                    for h in range(nquart):
                        sps = psB.tile([128, QW], f32, tag="sps")
                        for sub in range(QW // 512):
                            oo = slice(512 * sub, 512 * (sub + 1))
                            so = slice(QW * h + 512 * sub,
                                       QW * h + 512 * (sub + 1))
                            nc.tensor.matmul(out=sps[:, oo], lhsT=hiq[:, tl],
                                             rhs=hik[:, so],
                                             start=True, stop=False)
                            nc.tensor.matmul(out=sps[:, oo], lhsT=hiq[:, tl],
                                             rhs=lok[:, so],
                                             start=False, stop=False)
                            nc.tensor.matmul(out=sps[:, oo], lhsT=loq[:, tl],
                                             rhs=hik[0:C, so],
                                             start=False, stop=True)
                        nc.scalar.activation(out=ssb[:, QW * h:QW * (h + 1)],
                                             in_=sps[:], func=AF.Copy)

                  for t in range(NT):
                    tl = slice(128 * t, 128 * (t + 1))
                    ssb = ssbp.tile([128, N], f32, tag="ssb")
                    emit_scores(t, ssb)
                    cand = workp.tile([128, 8 * NCH], f32, tag="cand")
                    for ch in range(NCH):
                        nc.vector.max(out=cand[:, 8 * ch:8 * ch + 8],
                                      in_=ssb[:, CHK * ch:CHK * (ch + 1)])
                    t16 = workp.tile([128, 16], f32, tag="t16")
                    cand2 = workp.tile([128, 8 * NCH], f32, tag="cand2")
                    nc.vector.max(out=t16[:, 0:8], in_=cand[:])
                    nc.vector.match_replace(out=cand2[:], in_to_replace=t16[:, 0:8],
                                            in_values=cand[:], imm_value=NEG)
                    nc.vector.max(out=t16[:, 8:16], in_=cand2[:])
                    idxs = workp.tile([128, 16], u32, tag="idxs")
                    nc.vector.max_index(out=idxs[:, 0:8], in_max=t16[:, 0:8],
                                        in_values=ssb[:])
                    nc.vector.max_index(out=idxs[:, 8:16], in_max=t16[:, 8:16],
                                        in_values=ssb[:])
                    # 16 indirect gathers (vector-indirect: 1 offset/partition)
                    g = gp.tile([128, KNB, O], f32, tag="g")
                    for k in range(KNB):
                        nc.gpsimd.indirect_dma_start(
                            out=g[:, k, :], out_offset=None, in_=vtab[l].ap(),
                            in_offset=bass.IndirectOffsetOnAxis(
                                ap=idxs[:, k:k + 1], axis=0))
                    g2 = gp.tile([128, KNB, O], f32, tag="g2")
                    nc.scalar.activation(out=g2[:], in_=g[:], func=AF.Square)
                    nc.vector.tensor_tensor(out=qa[:], in0=qa[:], in1=g2[:],
                                            op=AO.add)
                    nc.vector.tensor_tensor(out=s1a[:], in0=s1a[:], in1=g[:],
                                            op=AO.add)
                    ubc = ubuf[:, t, :]
                    ubc = bass.AP(ubc.tensor, ubc.offset,
                                  [ubc.ap[0], [0, KNB], ubc.ap[-1]])
                    ug16 = ugp.tile([128, KNB, O], f32, tag="ug16")
                    nc.vector.tensor_tensor(out=ug16[:], in0=g[:, :, :],
                                            in1=ubc, op=AO.mult)
                    nc.vector.tensor_tensor(out=uga[:], in0=uga[:], in1=ug16[:],
                                            op=AO.add)
                    # max tree in place on g; z = u + max_k v into Dbuf
                    nc.vector.tensor_tensor(out=g[:, 0:8, :], in0=g[:, 0:8, :],
                                            in1=g[:, 8:16, :], op=AO.max)
                    for hh in (4, 2):
                        nc.vector.tensor_tensor(
                            out=g[:, 0:hh, :], in0=g[:, 0:hh, :],
                            in1=g[:, hh:2 * hh, :], op=AO.max)
                    nc.vector.tensor_tensor(out=Dbuf[:, t, :], in0=g[:, 0, :],
                                            in1=g[:, 1, :], op=AO.max)
                    nc.vector.tensor_tensor(out=Dbuf[:, t, :], in0=Dbuf[:, t, :],
                                            in1=ubuf[:, t, :], op=AO.add)
                # --- phase C: stats, allreduce (overlapped with transposes) ---
                # fold accumulators over k; build S1/S2; reduce over p via PE
                for hh in (8, 4, 2):
                    for acc in (s1a, qa, uga):
                        nc.vector.tensor_tensor(
                            out=acc[:, 0:hh, :], in0=acc[:, 0:hh, :],
                            in1=acc[:, hh:2 * hh, :], op=AO.add)
                # S1 = Gv + 16u (into s1a[:,0,:] + s1a[:,1,:] pre-fold)
                s1f = workp.tile([128, O], f32, tag="s1f")
                nc.vector.tensor_tensor(out=s1f[:], in0=s1a[:, 0, :],
                                        in1=s1a[:, 1, :], op=AO.add)
                nc.vector.scalar_tensor_tensor(
                    out=s1f[:], in0=usum[:, 0, :], scalar=16.0,
                    in1=s1f[:], op0=AO.mult, op1=AO.add)
                # S2 = Gss + 2*uGv + 16*usq
                s2f = workp.tile([128, O], f32, tag="s2f")
                nc.vector.tensor_tensor(out=s2f[:], in0=qa[:, 0, :],
                                        in1=qa[:, 1, :], op=AO.add)
                ugf = workp.tile([128, O], f32, tag="ugf")
                nc.vector.tensor_tensor(out=ugf[:], in0=uga[:, 0, :],
                                        in1=uga[:, 1, :], op=AO.add)
                nc.vector.scalar_tensor_tensor(
                    out=s2f[:], in0=ugf[:], scalar=2.0,
                    in1=s2f[:], op0=AO.mult, op1=AO.add)
                nc.vector.scalar_tensor_tensor(
                    out=s2f[:], in0=usq[:, 0, :], scalar=16.0,
                    in1=s2f[:], op0=AO.mult, op1=AO.add)
                with tc.tile_pool(name=f"psR{l}", bufs=1, space="PSUM") as psR:
                    s_ps = psR.tile([O, 2], f32, tag="sps2")
                    nc.tensor.matmul(out=s_ps[:, 0:1], lhsT=s1f[:],
                                     rhs=ones128[:], start=True, stop=True)
                    nc.tensor.matmul(out=s_ps[:, 1:2], lhsT=s2f[:],
                                     rhs=ones128[:], start=True, stop=True)
                    stg = workp.tile([O, 2], f32, tag="stg")
                    nc.vector.tensor_copy(out=stg[:], in_=s_ps[:])
                    nc.sync.dma_start(out=cc_in[l].ap(), in_=stg[:])
                if sim_single or not use_collective:
                    nc.sync.dma_start(out=cc_out[l].ap(), in_=cc_in[l].ap())
                else:
                    nc.gpsimd.collective_compute(
                        "AllReduce", AO.add, replica_groups=[list(range(NCORES))],
                        ins=[cc_in[l].ap()], outs=[cc_out[l].ap()])

                # transposes into xnq overlap the collective (plain copies)
                xnq = bigp.tile([O3 if is_last else O, N], f32, tag="xq")
                with tc.tile_pool(name=f"psT{l}", bufs=4, space="PSUM") as psT:
                    for t in range(NT):
                        tl = slice(128 * t, 128 * (t + 1))
                        tp = psT.tile([O, 128], f32, tag="tpps")
                        nc.tensor.transpose(out=tp[:], in_=Dbuf[:, t, :],
                                            identity=ident[:])
                        nc.scalar.activation(out=xnq[0:O, tl], in_=tp[:],
                                             func=AF.Copy)

                # stats arrive: finish BN affine coefficients
                stats = workp.tile([O, 2], f32, tag="stats")
                nc.sync.dma_start(out=stats[:], in_=cc_out[l].ap())
                mean = workp.tile([O, 4], f32, tag="mean")
                nc.vector.tensor_scalar(mean[:, 0:1], stats[:, 0:1], 1.0 / NTOT,
                                        None, op0=AO.mult)
                nc.vector.tensor_scalar(mean[:, 1:2], stats[:, 1:2], 1.0 / NTOT,
                                        None, op0=AO.mult)
                # var = E2 - mean^2 ; sd = sqrt(var+eps); a = gamma/sd; b = beta - mean*a
                nc.vector.tensor_tensor(out=mean[:, 2:3], in0=mean[:, 0:1],
                                        in1=mean[:, 0:1], op=AO.mult)
                nc.vector.tensor_sub(mean[:, 1:2], mean[:, 1:2], mean[:, 2:3])
                nc.scalar.activation(out=mean[:, 1:2], in_=mean[:, 1:2],
                                     func=AF.Sqrt, bias=eps128[0:O, :])
                ab = workp.tile([O, 2], f32, tag="ab")
                nc.vector.reciprocal(out=ab[:, 0:1], in_=mean[:, 1:2])
                nc.vector.tensor_tensor(out=ab[:, 0:1], in0=ab[:, 0:1],
                                        in1=gb[:, 0:1], op=AO.mult)
                nc.vector.tensor_tensor(out=mean[:, 3:4], in0=mean[:, 0:1],
                                        in1=ab[:, 0:1], op=AO.mult)
                nc.vector.tensor_sub(ab[:, 1:2], gb[:, 1:2], mean[:, 3:4])

                # x_next = relu(a*(u+D) + b), column-chunked so next-layer
                # phase A pipelines behind it
                for qh in range(4):
                    qs = slice(1024 * qh, 1024 * (qh + 1))
                    nc.scalar.activation(out=xnq[0:O, qs], in_=xnq[0:O, qs],
                                         func=AF.Relu,
                                         bias=ab[:, 1:2], scale=ab[:, 0:1])
                # channel max pool -> xg slice
                cm = workp.tile([O, 1], f32, tag="cm")
                nc.vector.tensor_reduce(out=cm[:], in_=xnq[0:O, :],
                                        axis=mybir.AxisListType.X, op=AO.max)
                off = {1: 0, 2: O1, 3: O1 + O2}[l]
                nc.sync.dma_start(out=xg[off:off + O, :], in_=cm[:])
                return xnq

            # layer 1 input
            xq1 = bigp.tile([C0, N], f32, tag="xq")
            nc.sync.dma_start(out=xq1[0:C0, :], in_=x_in.ap())

            xq2 = layer(1, C0, O1, xq1, False)
            xq3 = layer(2, O1, O2, xq2, False)
            layer(3, O2, O3, xq3, True)

            # FC: out = xg^T @ WfcT + bfc
            wfct = constp.tile([128, 64], f32)
            nc.sync.dma_start(out=wfct[:], in_=wfct_in.ap())
            bfc = constp.tile([1, 64], f32)
            nc.sync.dma_start(out=bfc[:], in_=bfc_in.ap())
            with tc.tile_pool(name="psF", bufs=1, space="PSUM") as psF:
                fc_ps = psF.tile([1, 64], f32, tag="fcps")
                nc.tensor.matmul(out=fc_ps[:], lhsT=xg[:], rhs=wfct[:],
                                 start=True, stop=True)
                ores = constp.tile([1, 64], f32)
                nc.vector.tensor_add(ores[:], fc_ps[:], bfc[:])
                nc.sync.dma_start(out=out_d.ap(), in_=ores[:])

    nc.compile()
    return nc


def _get_nc():
    if "nc" not in _cache:
        _cache["nc"] = _build()
    return _cache["nc"]


def _bf16_dtype():
    from ml_dtypes import bfloat16  # ships with jax
    return bfloat16


def _prep_inputs(x, W1, g1, b1, W2, g2, b2, W3, g3, b3, Wfc, bfc):
    """Host-side weight prep -> per-core input maps."""
    def wuv(W, C):
        A, Bm = W[:, :C], W[:, C:]
        return np.concatenate([(A - Bm).T, Bm.T], axis=1).astype(np.float32)

    common = {
        "wuv1": wuv(np.asarray(W1), C0),
        "wuv2": wuv(np.asarray(W2), O1),
        "wuv3": wuv(np.asarray(W3), O2),
        "gb1": np.stack([g1, b1], 1).astype(np.float32),
        "gb2": np.stack([g2, b2], 1).astype(np.float32),
        "gb3": np.stack([g3, b3], 1).astype(np.float32),
        "wfct": np.asarray(Wfc).T.copy().astype(np.float32),
        "onesbf": np.ones((1, N), np.float32).astype(_bf16_dtype()),
        "bfc": np.asarray(bfc)[None, :].astype(np.float32),
    }
    x = np.asarray(x, dtype=np.float32)
    return [{**common, "x": np.ascontiguousarray(x[c])} for c in range(NCORES)]


def _enable_jax_cache():
    try:
        import jax
        jax.config.update("jax_compilation_cache_dir", "/tmp/jaxcache")
        jax.config.update("jax_persistent_cache_min_entry_size_bytes", -1)
        jax.config.update("jax_persistent_cache_min_compile_time_secs", 0.5)
    except Exception:
        pass


def kernel(x, W1, g1, b1, W2, g2, b2, W3, g3, b3, Wfc, bfc):
    from concourse.bass_utils import run_bass_kernel_spmd
    _enable_jax_cache()
    nc = _get_nc()
    in_maps = _prep_inputs(x, W1, g1, b1, W2, g2, b2, W3, g3, b3, Wfc, bfc)
    res = run_bass_kernel_spmd(nc, in_maps, list(range(NCORES)))
    return np.stack([res.results[c]["out"][0] for c in range(NCORES)]).astype(np.float32)


# revision 34
# speedup vs baseline: 4.8227x; 4.6078x over previous
"""DGCNN feature extractor on 8 Trainium2 NeuronCores (Bass/Tile).

Strategy: data-parallel over batch B=8 (one sample per core).
Per layer (edge-conv):
  - scores s[n,m] = <x_n, x_m> - |x_m|^2/2 (rank-equivalent to the
    reference's -pairwise-distance) via THREE bf16 matmuls per 1024-col
    chunk using a hi/lo bf16 split (x = hi + lo):
    s = hiq.hik + hiq.lok + loq.hik accumulated in fp32 PSUM, where the
    q-side aug row is ones (lo = 0, term dropped) and the k-side aug row
    is the hi/lo split of -|x_m|^2/2. Empirically (numpy study) this
    keeps final rel err ~1.4e-3 (vs 2e-2 budget); single-bf16 scores or
    reduced-precision selection fail. bf16 matmul streams 1 col/cycle
    vs fp32's 4, cutting PE time ~2.3x.
  - selection stays fp32: PSUM copied to a full-width fp32 SBUF row;
    chunked max8 (8x512) -> top-16 of 64 candidates -> max_index over
    the fp32 row for global indices.
  - conv decomposes as y[o,n,k] = u[o,n] + v[o, idx[n,k]] with
    u = (A-B)x, v = Bx (W = [A|B] split, host-prepped); 16 per-k
    vector-indirect DMAs gather v rows (SWDGE supports one offset per
    partition per instruction - batched offsets verified broken on HW).
    GpSimd does ONLY gathers + collectives: all other work was moved to
    PE/DVE/ACT so the gather descriptor generation (~1.32us x 16/tile),
    which is the hard architectural floor here, is the sole GpSimd load.
  - k-folds are contiguous in-place log-trees (max on g, sum on a small
    scratch, sum-sq on g2), replacing 4x-slower strided reduces.
  - BN batch stats fused per tile into two SBUF accumulators:
    s1 += 16u + sum_k v ; s2 += 16u^2 + 2u*sum_k v + sum_k v^2, then one
    PE ones-matmul pair -> [O,2] -> AllReduce across the 8 cores while
    the (u+D) transposes run; x_next = relu(a*(u + max_k v) + b) since
    ReLU(LeakyReLU(z)) = ReLU(z) and the BN scale is positive.
Final: per-channel max over points, concat 32+32+64, FC on device, host
stacks the 8 per-core [64] outputs.
"""
import numpy as np

B, C0, N, KNB = 8, 3, 4096, 16
O1, O2, O3 = 32, 32, 64
NCORES = 8
EPS = 1e-5
NTOT = float(B * N * KNB)
NT = N // 128          # 32 point-tiles per layer
NCH = 8                # score chunks per row (4096/512)
CHK = N // NCH         # 512
NEG = -3.0e38

_cache: dict = {}


def _build(sim_single=False, use_collective=True, ssb_bufs=3, nquart=4):
    import concourse.bacc as bacc
    import concourse.bass as bass
    import concourse.mybir as mybir
    import concourse.tile as tile
    from concourse.masks import make_identity

    f32 = mybir.dt.float32
    bf16 = mybir.dt.bfloat16
    u32 = mybir.dt.uint32
    AO = mybir.AluOpType
    AF = mybir.ActivationFunctionType

    nc = bacc.Bacc("TRN2", target_bir_lowering=False, debug=False,
                   num_devices=1 if sim_single else NCORES)

    # ---- I/O ----
    x_in = nc.dram_tensor("x", [C0, N], f32, kind="ExternalInput")
    wuv_in = [None,
              nc.dram_tensor("wuv1", [C0, 2 * O1], f32, kind="ExternalInput"),
              nc.dram_tensor("wuv2", [O1, 2 * O2], f32, kind="ExternalInput"),
              nc.dram_tensor("wuv3", [O2, 2 * O3], f32, kind="ExternalInput")]
    gb_in = [None,
             nc.dram_tensor("gb1", [O1, 2], f32, kind="ExternalInput"),
             nc.dram_tensor("gb2", [O2, 2], f32, kind="ExternalInput"),
             nc.dram_tensor("gb3", [O3, 2], f32, kind="ExternalInput")]
    wfct_in = nc.dram_tensor("wfct", [128, 64], f32, kind="ExternalInput")
    bfc_in = nc.dram_tensor("bfc", [1, 64], f32, kind="ExternalInput")
    onesbf_in = nc.dram_tensor("onesbf", [1, N], mybir.dt.bfloat16,
                               kind="ExternalInput")
    out_d = nc.dram_tensor("out", [1, 64], f32, kind="ExternalOutput")

    # ---- internal DRAM ----
    vtab = [None,
            nc.dram_tensor("vtab1", [N, O1], f32),
            nc.dram_tensor("vtab2", [N, O2], f32),
            nc.dram_tensor("vtab3", [N, O3], f32)]
    cc_in = [None] + [nc.dram_tensor(f"ccin{l}", [o, 2], f32)
                      for l, o in ((1, O1), (2, O2), (3, O3))]
    cc_out = [None] + [nc.dram_tensor(f"ccout{l}", [o, 2], f32,
                                      addr_space="Shared")
                       for l, o in ((1, O1), (2, O2), (3, O3))]

    QW = N // nquart       # 1024 columns per score chunk

    with tile.TileContext(nc) as tc:
        with (
            tc.tile_pool(name="big", bufs=2) as bigp,        # xq generations
            tc.tile_pool(name="lay", bufs=1) as layp,        # per-layer buffers
            tc.tile_pool(name="work", bufs=3) as workp,      # small per-tile tiles
            tc.tile_pool(name="ssb", bufs=ssb_bufs) as ssbp, # SBUF score rows
            tc.tile_pool(name="gpool", bufs=3) as gp,
            tc.tile_pool(name="ugp", bufs=2) as ugp,        # gather tiles
            tc.tile_pool(name="const", bufs=1) as constp,
        ):
            ident = constp.tile([128, 128], f32)
            make_identity(nc, ident[:])
            ones128 = constp.tile([128, 1], f32)
            nc.vector.memset(ones128[:], 1.0)
            zero128 = constp.tile([128, 1], f32)
            nc.vector.memset(zero128[:], 0.0)
            eps128 = constp.tile([128, 1], f32)
            nc.vector.memset(eps128[:], EPS)
            xg = constp.tile([128, 1], f32)   # pooled channel maxes (x1|x2|x3)

            def layer(l, C, O, xq, is_last):
                """xq: [C, N] fp32 SBUF tile (features only).
                Returns next layer's xq ([O, N] fp32) or None if is_last."""
                # --- phase A: sq row, bf16 hi/lo gens, u/v matmuls ---
                wuv = constp.tile([C, 2 * O], f32, tag=f"wuv{l}")
                nc.sync.dma_start(out=wuv[:], in_=wuv_in[l].ap())
                gb = constp.tile([O, 2], f32, tag=f"gb{l}")
                nc.sync.dma_start(out=gb[:], in_=gb_in[l].ap())
                onesC = constp.tile([C, 1], f32, tag=f"onesC{l}")
                nc.vector.memset(onesC[:], 1.0)

                ubuf = layp.tile([128, NT, O], f32, tag="ubuf")
                # bf16 hi/lo. q: rows 0..C-1 features, row C ones (lo term
                # dropped since lo(ones)=0). k: row C = hi/lo of -|x_m|^2/2.
                hiq = layp.tile([C + 1, N], bf16, tag="hiq")
                loq = layp.tile([C, N], bf16, tag="loq")
                hik = layp.tile([C + 1, N], bf16, tag="hik")
                lok = layp.tile([C + 1, N], bf16, tag="lok")

                with tc.tile_pool(name=f"psA{l}", bufs=2, space="PSUM") as psA:
                    for ch in range(8):
                        sl = slice(512 * ch, 512 * (ch + 1))
                        xsq = ugp.tile([C, 512], f32, tag="xsq")
                        nc.scalar.activation(out=xsq[:], in_=xq[0:C, sl],
                                             func=AF.Square, bias=zero128[0:C, :])
                        sq_ps = psA.tile([1, 512], f32, tag="sqps")
                        nc.tensor.matmul(out=sq_ps[:], lhsT=onesC[:],
                                         rhs=xsq[:], start=True, stop=True)
                        sqf = ugp.tile([1, 512], f32, tag="sqf")
                        nc.scalar.activation(out=sqf[:], in_=sq_ps[:],
                                             func=AF.Copy, scale=-0.5)
                        sqh = workp.tile([1, 512], bf16, tag="sqh")
                        nc.scalar.activation(out=sqh[:], in_=sqf[:],
                                             func=AF.Copy)
                        sql = workp.tile([1, 512], bf16, tag="sql")
                        nc.vector.tensor_sub(sql[:], sqf[:], sqh[:])
                        nc.sync.dma_start(out=hik[C:C + 1, sl], in_=sqh[:])
                        nc.sync.dma_start(out=lok[C:C + 1, sl], in_=sql[:])
                    # hi/lo feature rows (ACT cast + DVE subtract), in
                    # column chunks so early score quarters start sooner
                    nc.sync.dma_start(out=hiq[C:C + 1, :], in_=onesbf_in.ap())
                    for qh in range(4):
                        qs = slice(1024 * qh, 1024 * (qh + 1))
                        nc.scalar.activation(out=hiq[0:C, qs], in_=xq[0:C, qs],
                                             func=AF.Copy)
                        nc.vector.tensor_sub(loq[0:C, qs], xq[0:C, qs],
                                             hiq[0:C, qs])
                        nc.sync.dma_start(out=hik[0:C, qs], in_=hiq[0:C, qs])
                        nc.sync.dma_start(out=lok[0:C, qs], in_=loq[0:C, qs])

                    for tu in range(NT):
                        tlu = slice(128 * tu, 128 * (tu + 1))
                        uv_ps = psA.tile([128, 2 * O], f32, tag="uvps")
                        nc.tensor.matmul(out=uv_ps[:], lhsT=xq[0:C, tlu],
                                         rhs=wuv[:], start=True, stop=True)
                        nc.scalar.activation(out=ubuf[:, tu, :],
                                             in_=uv_ps[:, 0:O], func=AF.Copy)
                        vstage = workp.tile([128, O], f32, tag="vstage")
                        nc.scalar.activation(out=vstage[:],
                                             in_=uv_ps[:, O:2 * O],
                                             func=AF.Copy)
                        nc.sync.dma_start(out=vtab[l].ap()[tlu, :],
                                          in_=vstage[:])


                # u and u^2 sums over tiles (needs only ubuf; overlaps phase B)
                usum = layp.tile([128, NT // 2, O], f32, tag="usum")
                nc.vector.tensor_tensor(out=usum[:], in0=ubuf[:, 0:NT // 2, :],
                                        in1=ubuf[:, NT // 2:NT, :], op=AO.add)
                usq = layp.tile([128, NT // 2, O], f32, tag="usq")
                nc.scalar.activation(out=usq[:], in_=ubuf[:, 0:NT // 2, :],
                                     func=AF.Square)
                usq2 = layp.tile([128, NT // 2, O], f32, tag="usq2")
                nc.scalar.activation(out=usq2[:], in_=ubuf[:, NT // 2:NT, :],
                                     func=AF.Square)
                nc.vector.tensor_tensor(out=usq[:], in0=usq[:], in1=usq2[:],
                                        op=AO.add)
                for hh in (8, 4, 2, 1):
                    nc.vector.tensor_tensor(out=usum[:, 0:hh, :],
                                            in0=usum[:, 0:hh, :],
                                            in1=usum[:, hh:2 * hh, :], op=AO.add)
                    nc.vector.tensor_tensor(out=usq[:, 0:hh, :],
                                            in0=usq[:, 0:hh, :],
                                            in1=usq[:, hh:2 * hh, :], op=AO.add)

                # BN stat accumulators (summed over tiles, fp32, k-resolved)
                s1a = layp.tile([128, KNB, O], f32, tag="s1a")
                qa = layp.tile([128, KNB, O], f32, tag="qa")
                uga = layp.tile([128, KNB, O], f32, tag="uga")
                nc.vector.memset(s1a[:], 0.0)
                nc.vector.memset(qa[:], 0.0)
                nc.vector.memset(uga[:], 0.0)

                # --- phase B: scores + topk + gather + folds ---
                # (tile 0's score matmuls emitted before the u/v matmuls so
                # the PE pipeline restarts immediately at the layer boundary;
                # all vtab rows still land before the first gather)
                Dbuf = layp.tile([128, NT, O], f32, tag="Dbuf")
                with tc.tile_pool(name=f"psB{l}", bufs=2, space="PSUM") as psB:
                  def emit_scores(t, ssb):
                    tl = slice(128 * t, 128 * (t + 1))
                    for h in range(nquart):
                        sps = psB.tile([128, QW], f32, tag="sps")
                        for sub in range(QW // 512):
                            oo = slice(512 * sub, 512 * (sub + 1))
                            so = slice(QW * h + 512 * sub,
                                       QW * h + 512 * (sub + 1))
                            nc.tensor.matmul(out=sps[:, oo], lhsT=hiq[:, tl],
                                             rhs=hik[:, so],
                                             start=True, stop=False)
                            nc.tensor.matmul(out=sps[:, oo], lhsT=hiq[:, tl],
                                             rhs=lok[:, so],
                                             start=False, stop=False)
                            nc.tensor.matmul(out=sps[:, oo], lhsT=loq[:, tl],
                                             rhs=hik[0:C, so],
                                             start=False, stop=True)
                        nc.scalar.activation(out=ssb[:, QW * h:QW * (h + 1)],
                                             in_=sps[:], func=AF.Copy)

                  for t in range(NT):
                    tl = slice(128 * t, 128 * (t + 1))
                    ssb = ssbp.tile([128, N], f32, tag="ssb")
                    emit_scores(t, ssb)
                    cand = workp.tile([128, 8 * NCH], f32, tag="cand")
                    for ch in range(NCH):
                        nc.vector.max(out=cand[:, 8 * ch:8 * ch + 8],
                                      in_=ssb[:, CHK * ch:CHK * (ch + 1)])
                    t16 = workp.tile([128, 16], f32, tag="t16")
                    cand2 = workp.tile([128, 8 * NCH], f32, tag="cand2")
                    nc.vector.max(out=t16[:, 0:8], in_=cand[:])
                    nc.vector.match_replace(out=cand2[:], in_to_replace=t16[:, 0:8],
                                            in_values=cand[:], imm_value=NEG)
                    nc.vector.max(out=t16[:, 8:16], in_=cand2[:])
                    idxs = workp.tile([128, 16], u32, tag="idxs")
                    nc.vector.max_index(out=idxs[:, 0:8], in_max=t16[:, 0:8],
                                        in_values=ssb[:])
                    nc.vector.max_index(out=idxs[:, 8:16], in_max=t16[:, 8:16],
                                        in_values=ssb[:])
                    # 16 indirect gathers (vector-indirect: 1 offset/partition)
                    g = gp.tile([128, KNB, O], f32, tag="g")
                    for k in range(KNB):
                        nc.gpsimd.indirect_dma_start(
                            out=g[:, k, :], out_offset=None, in_=vtab[l].ap(),
                            in_offset=bass.IndirectOffsetOnAxis(
                                ap=idxs[:, k:k + 1], axis=0))
                    g2 = gp.tile([128, KNB, O], f32, tag="g2")
                    nc.scalar.activation(out=g2[:], in_=g[:], func=AF.Square)
                    nc.vector.tensor_tensor(out=qa[:], in0=qa[:], in1=g2[:],
                                            op=AO.add)
                    nc.vector.tensor_tensor(out=s1a[:], in0=s1a[:], in1=g[:],
                                            op=AO.add)
                    ubc = ubuf[:, t, :]
                    ubc = bass.AP(ubc.tensor, ubc.offset,
                                  [ubc.ap[0], [0, KNB], ubc.ap[-1]])
                    ug16 = ugp.tile([128, KNB, O], f32, tag="ug16")
                    nc.vector.tensor_tensor(out=ug16[:], in0=g[:, :, :],
                                            in1=ubc, op=AO.mult)
                    nc.vector.tensor_tensor(out=uga[:], in0=uga[:], in1=ug16[:],
                                            op=AO.add)
                    # max tree in place on g; z = u + max_k v into Dbuf
                    nc.vector.tensor_tensor(out=g[:, 0:8, :], in0=g[:, 0:8, :],
                                            in1=g[:, 8:16, :], op=AO.max)
                    for hh in (4, 2):
                        nc.vector.tensor_tensor(
                            out=g[:, 0:hh, :], in0=g[:, 0:hh, :],
                            in1=g[:, hh:2 * hh, :], op=AO.max)
                    nc.vector.tensor_tensor(out=Dbuf[:, t, :], in0=g[:, 0, :],
                                            in1=g[:, 1, :], op=AO.max)
                    nc.vector.tensor_tensor(out=Dbuf[:, t, :], in0=Dbuf[:, t, :],
                                            in1=ubuf[:, t, :], op=AO.add)
                # --- phase C: stats, allreduce (overlapped with transposes) ---
                # fold accumulators over k; build S1/S2; reduce over p via PE
                for hh in (8, 4, 2):
                    for acc in (s1a, qa, uga):
                        nc.vector.tensor_tensor(
                            out=acc[:, 0:hh, :], in0=acc[:, 0:hh, :],
                            in1=acc[:, hh:2 * hh, :], op=AO.add)
                # S1 = Gv + 16u (into s1a[:,0,:] + s1a[:,1,:] pre-fold)
                s1f = workp.tile([128, O], f32, tag="s1f")
                nc.vector.tensor_tensor(out=s1f[:], in0=s1a[:, 0, :],
                                        in1=s1a[:, 1, :], op=AO.add)
                nc.vector.scalar_tensor_tensor(
                    out=s1f[:], in0=usum[:, 0, :], scalar=16.0,
                    in1=s1f[:], op0=AO.mult, op1=AO.add)
                # S2 = Gss + 2*uGv + 16*usq
                s2f = workp.tile([128, O], f32, tag="s2f")
                nc.vector.tensor_tensor(out=s2f[:], in0=qa[:, 0, :],
                                        in1=qa[:, 1, :], op=AO.add)
                ugf = workp.tile([128, O], f32, tag="ugf")
                nc.vector.tensor_tensor(out=ugf[:], in0=uga[:, 0, :],
                                        in1=uga[:, 1, :], op=AO.add)
                nc.vector.scalar_tensor_tensor(
                    out=s2f[:], in0=ugf[:], scalar=2.0,
                    in1=s2f[:], op0=AO.mult, op1=AO.add)
                nc.vector.scalar_tensor_tensor(
                    out=s2f[:], in0=usq[:, 0, :], scalar=16.0,
                    in1=s2f[:], op0=AO.mult, op1=AO.add)
                with tc.tile_pool(name=f"psR{l}", bufs=1, space="PSUM") as psR:
                    s_ps = psR.tile([O, 2], f32, tag="sps2")
                    nc.tensor.matmul(out=s_ps[:, 0:1], lhsT=s1f[:],
                                     rhs=ones128[:], start=True, stop=True)
                    nc.tensor.matmul(out=s_ps[:, 1:2], lhsT=s2f[:],
                                     rhs=ones128[:], start=True, stop=True)
                    stg = workp.tile([O, 2], f32, tag="stg")
                    nc.vector.tensor_copy(out=stg[:], in_=s_ps[:])
                    nc.sync.dma_start(out=cc_in[l].ap(), in_=stg[:])
                if sim_single or not use_collective:
                    nc.sync.dma_start(out=cc_out[l].ap(), in_=cc_in[l].ap())
                else:
                    nc.gpsimd.collective_compute(
                        "AllReduce", AO.add, replica_groups=[list(range(NCORES))],
                        ins=[cc_in[l].ap()], outs=[cc_out[l].ap()])

                # transposes into xnq overlap the collective (plain copies).
                # Last layer: only the per-channel max survives, and the BN
                # scale is positive, so fold the max over points BEFORE the
                # affine: one tree + one transpose instead of 32.
                if is_last:
                    zfold = usq2  # dead after the u^2 tree; same shape
                    nc.vector.tensor_tensor(out=zfold[:],
                                            in0=Dbuf[:, 0:NT // 2, :],
                                            in1=Dbuf[:, NT // 2:NT, :],
                                            op=AO.max)
                    for hh in (8, 4, 2, 1):
                        nc.vector.tensor_tensor(out=zfold[:, 0:hh, :],
                                                in0=zfold[:, 0:hh, :],
                                                in1=zfold[:, hh:2 * hh, :],
                                                op=AO.max)
                    xnq = None
                else:
                    xnq = bigp.tile([O, N], f32, tag="xq")
                with tc.tile_pool(name=f"psT{l}", bufs=4, space="PSUM") as psT:
                    if is_last:
                        tpz = psT.tile([O, 128], f32, tag="tpps")
                        nc.tensor.transpose(out=tpz[:], in_=zfold[:, 0, :],
                                            identity=ident[:])
                        zmt = workp.tile([O, 128], f32, tag="zmt")
                        nc.vector.tensor_copy(out=zmt[:], in_=tpz[:])
                    else:
                        for t in range(NT):
                            tl = slice(128 * t, 128 * (t + 1))
                            tp = psT.tile([O, 128], f32, tag="tpps")
                            nc.tensor.transpose(out=tp[:], in_=Dbuf[:, t, :],
                                                identity=ident[:])
                            nc.scalar.activation(out=xnq[0:O, tl], in_=tp[:],
                                                 func=AF.Copy)

                # stats arrive: finish BN affine coefficients
                stats = workp.tile([O, 2], f32, tag="stats")
                nc.sync.dma_start(out=stats[:], in_=cc_out[l].ap())
                mean = workp.tile([O, 4], f32, tag="mean")
                nc.vector.tensor_scalar(mean[:, 0:1], stats[:, 0:1], 1.0 / NTOT,
                                        None, op0=AO.mult)
                nc.vector.tensor_scalar(mean[:, 1:2], stats[:, 1:2], 1.0 / NTOT,
                                        None, op0=AO.mult)
                # var = E2 - mean^2 ; sd = sqrt(var+eps); a = gamma/sd; b = beta - mean*a
                nc.vector.tensor_tensor(out=mean[:, 2:3], in0=mean[:, 0:1],
                                        in1=mean[:, 0:1], op=AO.mult)
                nc.vector.tensor_sub(mean[:, 1:2], mean[:, 1:2], mean[:, 2:3])
                nc.scalar.activation(out=mean[:, 1:2], in_=mean[:, 1:2],
                                     func=AF.Sqrt, bias=eps128[0:O, :])
                ab = workp.tile([O, 2], f32, tag="ab")
                nc.vector.reciprocal(out=ab[:, 0:1], in_=mean[:, 1:2])
                nc.vector.tensor_tensor(out=ab[:, 0:1], in0=ab[:, 0:1],
                                        in1=gb[:, 0:1], op=AO.mult)
                nc.vector.tensor_tensor(out=mean[:, 3:4], in0=mean[:, 0:1],
                                        in1=ab[:, 0:1], op=AO.mult)
                nc.vector.tensor_sub(ab[:, 1:2], gb[:, 1:2], mean[:, 3:4])

                off = {1: 0, 2: O1, 3: O1 + O2}[l]
                cm = workp.tile([O, 1], f32, tag="cm")
                if is_last:
                    # xg slice = relu(a * max_n z + b)
                    zred = workp.tile([O, 1], f32, tag="zred")
                    nc.vector.tensor_reduce(out=zred[:], in_=zmt[:],
                                            axis=mybir.AxisListType.X,
                                            op=AO.max)
                    nc.scalar.activation(out=cm[:], in_=zred[:], func=AF.Relu,
                                         bias=ab[:, 1:2], scale=ab[:, 0:1])
                else:
                    # x_next = relu(a*(u+D) + b), column-chunked so next-layer
                    # phase A pipelines behind it
                    for qh in range(4):
                        qs = slice(1024 * qh, 1024 * (qh + 1))
                        nc.scalar.activation(out=xnq[0:O, qs],
                                             in_=xnq[0:O, qs], func=AF.Relu,
                                             bias=ab[:, 1:2], scale=ab[:, 0:1])
                    nc.vector.tensor_reduce(out=cm[:], in_=xnq[0:O, :],
                                            axis=mybir.AxisListType.X, op=AO.max)
                nc.sync.dma_start(out=xg[off:off + O, :], in_=cm[:])
                return xnq

            # layer 1 input
            xq1 = bigp.tile([C0, N], f32, tag="xq")
            nc.sync.dma_start(out=xq1[0:C0, :], in_=x_in.ap())

            xq2 = layer(1, C0, O1, xq1, False)
            xq3 = layer(2, O1, O2, xq2, False)
            layer(3, O2, O3, xq3, True)

            # FC: out = xg^T @ WfcT + bfc
            wfct = constp.tile([128, 64], f32)
            nc.sync.dma_start(out=wfct[:], in_=wfct_in.ap())
            bfc = constp.tile([1, 64], f32)
            nc.sync.dma_start(out=bfc[:], in_=bfc_in.ap())
            with tc.tile_pool(name="psF", bufs=1, space="PSUM") as psF:
                fc_ps = psF.tile([1, 64], f32, tag="fcps")
                nc.tensor.matmul(out=fc_ps[:], lhsT=xg[:], rhs=wfct[:],
                                 start=True, stop=True)
                ores = constp.tile([1, 64], f32)
                nc.vector.tensor_add(ores[:], fc_ps[:], bfc[:])
                nc.sync.dma_start(out=out_d.ap(), in_=ores[:])

    nc.compile()
    return nc


def _get_nc():
    if "nc" not in _cache:
        _cache["nc"] = _build()
    return _cache["nc"]


def _bf16_dtype():
    from ml_dtypes import bfloat16  # ships with jax
    return bfloat16


def _prep_inputs(x, W1, g1, b1, W2, g2, b2, W3, g3, b3, Wfc, bfc):
    """Host-side weight prep -> per-core input maps."""
    def wuv(W, C):
        A, Bm = W[:, :C], W[:, C:]
        return np.concatenate([(A - Bm).T, Bm.T], axis=1).astype(np.float32)

    common = {
        "wuv1": wuv(np.asarray(W1), C0),
        "wuv2": wuv(np.asarray(W2), O1),
        "wuv3": wuv(np.asarray(W3), O2),
        "gb1": np.stack([g1, b1], 1).astype(np.float32),
        "gb2": np.stack([g2, b2], 1).astype(np.float32),
        "gb3": np.stack([g3, b3], 1).astype(np.float32),
        "wfct": np.asarray(Wfc).T.copy().astype(np.float32),
        "onesbf": np.ones((1, N), np.float32).astype(_bf16_dtype()),
        "bfc": np.asarray(bfc)[None, :].astype(np.float32),
    }
    x = np.asarray(x, dtype=np.float32)
    return [{**common, "x": np.ascontiguousarray(x[c])} for c in range(NCORES)]


def _enable_jax_cache():
    try:
        import jax
        jax.config.update("jax_compilation_cache_dir", "/tmp/jaxcache")
        jax.config.update("jax_persistent_cache_min_entry_size_bytes", -1)
        jax.config.update("jax_persistent_cache_min_compile_time_secs", 0.5)
    except Exception:
        pass


def kernel(x, W1, g1, b1, W2, g2, b2, W3, g3, b3, Wfc, bfc):
    from concourse.bass_utils import run_bass_kernel_spmd
    _enable_jax_cache()
    nc = _get_nc()
    in_maps = _prep_inputs(x, W1, g1, b1, W2, g2, b2, W3, g3, b3, Wfc, bfc)
    res = run_bass_kernel_spmd(nc, in_maps, list(range(NCORES)))
    return np.stack([res.results[c]["out"][0] for c in range(NCORES)]).astype(np.float32)
